# revision 14
# baseline (speedup 1.0000x reference)
"""Trainium2 Bass kernel for single-head attention + output projection + residual.

Math per batch element b (N=2048, D=512, U=128):
    Q = x @ W_q; K = x @ W_k; V = x @ W_v
    S = Q @ K.T / sqrt(U); A = softmax(S, axis=-1)
    out = (A @ V) @ W_o + b_o + x

Distribution: data-parallel over batch — 8 batch elements, one per NeuronCore.

v3 schedule: deep software pipeline. All scores for a query-half are issued
back-to-back (ScalarE exp chases them through a 3-deep PSUM pool into a
deep SBUF e-tile pool); ctx accumulation runs afterwards in fp8 DoubleRow
(two key-blocks contracted per instruction: V pairs [128,2,U] x e pairs
[128,2,q]), so ctx costs half the tensor cycles of bf16. exp output is fp8
with a folded -1 bias shift (cancels in softmax normalization). Projections
are ordered KT0,QT0,QT1 first so exp starts ASAP. xT is loaded via
partition-split multi-queue DMAs from a DMA-friendly host layout. Bias +
residual are folded host-side (xb = x + b_o).

Softmax max-subtraction is skipped: scores/sqrt(U) are bounded (~±6) for any
well-scaled input; exp output with -1 shift stays under fp8e4's ±240 range.
"""

import numpy as np
import ml_dtypes

import concourse.bass as bass
import concourse.tile as tile
from concourse import bacc, mybir
from concourse.bass_utils import run_bass_kernel_spmd

N = 2048
D = 512
U = 128
NB = N // 128  # 16 query/key blocks
DC = D // 128  # 4 d-chunks
NS = N // 512  # 4 free-dim slices of 512
HQ = N // 2  # queries per half
INV_SQRT_U = 1.0 / np.sqrt(U)
EXP_SHIFT = -1.0  # exp(s - 1): cancels in normalization, keeps fp8e4 in range

F32 = mybir.dt.float32
BF16 = mybir.dt.bfloat16
FP8 = mybir.dt.float8e4

DR = mybir.MatmulPerfMode.DoubleRow


def build_attention_nc():
    nc = bacc.Bacc("TRN2", target_bir_lowering=False, debug=False)

    xb_ext = nc.declare_dram_parameter("xb", [N, D], F32, isOutput=False)
    # host layout [p, ns, c, n5]: per-partition 4KB contiguous runs per piece
    xT_ext = nc.declare_dram_parameter("xT", [128, NS * DC * 512], BF16, isOutput=False)
    wq_ext = nc.declare_dram_parameter("wq", [128, D], BF16, isOutput=False)
    wk_ext = nc.declare_dram_parameter("wk", [128, D], BF16, isOutput=False)
    wv_ext = nc.declare_dram_parameter("wv", [128, D], BF16, isOutput=False)
    wo_ext = nc.declare_dram_parameter("wo", [U, D], BF16, isOutput=False)
    out_ext = nc.declare_dram_parameter("out", [N, D], F32, isOutput=True)

    with tile.TileContext(nc) as tc:
        _build_body(nc, tc, xb_ext, xT_ext, wq_ext, wk_ext, wv_ext, wo_ext, out_ext)
    nc.compile()
    return nc


def _build_body(nc, tc, xb_ext, xT_ext, wq_ext, wk_ext, wv_ext, wo_ext, out_ext):
    from contextlib import ExitStack

    with ExitStack() as ctx:
        const = ctx.enter_context(tc.tile_pool(name="const", bufs=1))

        # ---- loads: weights first (they gate the first matmuls) ----
        wq_sb = const.tile([128, D], BF16)  # [d-within-chunk, (c u)]
        wk_sb = const.tile([128, D], BF16)
        wv_sb = const.tile([128, D], BF16)
        wo_sb = const.tile([U, D], BF16)
        nc.scalar.dma_start(wk_sb[:], wk_ext.ap())
        nc.scalar.dma_start(wq_sb[:], wq_ext.ap())
        nc.gpsimd.dma_start(wv_sb[:], wv_ext.ap())
        nc.gpsimd.dma_start(wo_sb[:], wo_ext.ap())

        ones_sb = const.tile([128, 1], BF16)
        nc.vector.memset(ones_sb[:], 1.0)
        eshift_sb = const.tile([128, 1], F32)
        nc.vector.memset(eshift_sb[:], EXP_SHIFT)
        # force the exp activation table load while DMAs are in flight
        scratch = const.tile([128, 1], F32)
        nc.scalar.activation(
            scratch[:], ones_sb[:], mybir.ActivationFunctionType.Exp, scale=1.0
        )

        # xT in SBUF as [p, ns, c, n5]; 8 partition-split piece DMAs spread
        # over three issue queues so arrival is multi-queue parallel
        xT_sb = const.tile([128, NS, DC, 512], BF16)
        xT_r = xT_ext.ap().rearrange("p (ns c n) -> p ns c n", ns=NS, c=DC)
        dma_engs = [nc.sync, nc.gpsimd, nc.scalar]
        k = 0
        for ns in range(NS):
            for ph in range(2):
                p0 = ph * 64
                dma_engs[k % 3].dma_start(
                    xT_sb[p0:p0 + 64, ns], xT_r[p0:p0 + 64, ns]
                )
                k += 1

        # xb = x + b_o precomputed on host; needed only for the epilogue.
        # Two partition-split pieces, issued after xT on the same queues.
        xb_sb = const.tile([128, NB, D], F32)
        xb_r = xb_ext.ap().rearrange("(nb p) d -> p nb d", p=128)
        nc.sync.dma_start(xb_sb[0:64], xb_r[0:64])
        nc.gpsimd.dma_start(xb_sb[64:128], xb_r[64:128])

        QT_sb = const.tile([U, N], BF16)
        KT_sb = const.tile([U, N], BF16)
        V_sb = const.tile([128, NB, U], FP8)  # kb-pair p at [:, 2p:2p+2, :]
        ctxT_sb = const.tile([U, N], BF16)
        Esum = [const.tile([128, HQ], BF16, name=f"esum_{h}") for h in range(2)]
        r_sb = const.tile([128, NB], F32)

        def proj_slice(pool, w_sb, oT, ns):
            ps = pool.tile([128, 512], F32, tag="ps", name=f"pp_{oT.tensor.name}_{ns}")
            for c in range(DC):
                nc.tensor.matmul(
                    ps[:],
                    lhsT=w_sb[:, c * 128:(c + 1) * 128],
                    rhs=xT_sb[:, ns, c, :],
                    start=(c == 0),
                    stop=(c == DC - 1),
                )
            nc.vector.tensor_copy(oT[:, ns * 512:(ns + 1) * 512], ps[:])

        def make_v(pool, kb):
            ps = pool.tile([128, 512], F32, tag="ps", name=f"v_{kb}")
            ns, n0 = divmod(kb * 128, 512)
            for c in range(DC):
                nc.tensor.matmul(
                    ps[:, 0:128],
                    lhsT=xT_sb[:, ns, c, n0:n0 + 128],
                    rhs=wv_sb[:, c * 128:(c + 1) * 128],
                    start=(c == 0),
                    stop=(c == DC - 1),
                )
            nc.vector.tensor_copy(V_sb[:, kb, :], ps[:, 0:128])

        e_t = [None] * NB  # paired e-tiles [128, 2, HQ] per half-pair

        # ---- phase 1: projections + all scores/exp (PSUM: pp 2 + sp 6 banks) ----
        with (
            tc.tile_pool(name="proj_ps", bufs=2, space="PSUM") as pp,
            tc.tile_pool(name="s_ps", bufs=3, space="PSUM") as sp,
            tc.tile_pool(name="e_sb", bufs=16) as ep,
        ):
            proj_slice(pp, wk_sb, KT_sb, 0)
            proj_slice(pp, wq_sb, QT_sb, 0)
            proj_slice(pp, wq_sb, QT_sb, 1)

            def scores_block(h, kb):
                pr = (h * NB + kb) // 2
                t = kb % 2
                q0 = h * HQ
                if t == 0:
                    e_t[pr] = ep.tile([128, 2, HQ], FP8, tag="e", name=f"e_{h}_{kb}")
                s_ps = sp.tile([128, HQ], F32, tag="s", name=f"s_{h}_{kb}")
                for j in range(2):
                    nc.tensor.matmul(
                        s_ps[:, j * 512:(j + 1) * 512],
                        lhsT=KT_sb[:, kb * 128:(kb + 1) * 128],
                        rhs=QT_sb[:, q0 + j * 512:q0 + (j + 1) * 512],
                        start=True,
                        stop=True,
                    )
                nc.scalar.activation(
                    e_t[pr][:, t, :],
                    s_ps[:],
                    mybir.ActivationFunctionType.Exp,
                    bias=eshift_sb[:],
                    scale=INV_SQRT_U,
                )
                if kb == 0:
                    nc.vector.tensor_copy(Esum[h][:], e_t[pr][:, t, :])
                else:
                    nc.vector.tensor_add(Esum[h][:], Esum[h][:], e_t[pr][:, t, :])

            for kb in range(4):
                scores_block(0, kb)
            proj_slice(pp, wk_sb, KT_sb, 1)
            for kb in range(4, 8):
                scores_block(0, kb)
            proj_slice(pp, wk_sb, KT_sb, 2)
            for kb in range(8, 12):
                scores_block(0, kb)
            proj_slice(pp, wk_sb, KT_sb, 3)
            for kb in range(12, 16):
                scores_block(0, kb)

            proj_slice(pp, wq_sb, QT_sb, 2)
            proj_slice(pp, wq_sb, QT_sb, 3)
            for kb in range(NB):
                make_v(pp, kb)

            for kb in range(NB):
                scores_block(1, kb)

        # ---- phase 2: fp8 DoubleRow ctx + epilogue (PSUM: cp 4 + dp 3 banks) ----
        with (
            tc.tile_pool(name="ctx_ps", bufs=4, space="PSUM") as cp,
            tc.tile_pool(name="d_ps", bufs=3, space="PSUM") as dp,
            tc.tile_pool(name="o_sb", bufs=4) as op,
        ):
            def ctx_half(h):
                return [
                    cp.tile([U, 512], F32, tag="ctx", name=f"ctx_ps_{h}_{j}")
                    for j in range(2)
                ]

            def ctx_mms(h, pair, ctx_ps):
                pr = (h * NB) // 2 + pair
                v2 = V_sb[:, 2 * pair:2 * pair + 2, :]
                for j in range(2):
                    nc.tensor.matmul(
                        ctx_ps[j][:],
                        lhsT=v2,
                        rhs=e_t[pr][:, :, j * 512:(j + 1) * 512],
                        start=(pair == 0),
                        stop=(pair == NB // 2 - 1),
                        perf_mode=DR,
                    )

            def rowsum(h):
                scol = dp.tile([128, 512], F32, tag="d", name=f"scol_{h}")
                for i in range(8):
                    nc.tensor.matmul(
                        scol[:, i:i + 1],
                        lhsT=Esum[h][:, i * 128:(i + 1) * 128],
                        rhs=ones_sb[:],
                        start=(i == 0),
                        stop=(i == 7),
                        skip_group_check=True,
                    )
                nc.vector.reciprocal(r_sb[:, h * 8:h * 8 + 8], scol[:, 0:8])

            def ctx_copy(h, j, ctx_ps):
                nc.vector.tensor_copy(
                    ctxT_sb[:, h * HQ + j * 512:h * HQ + (j + 1) * 512], ctx_ps[j][:]
                )

            def epilogue_qb(h, qb_local, store_eng):
                qb = h * 8 + qb_local
                y_ps = dp.tile([128, D], F32, tag="d", name=f"y_{qb}")
                nc.tensor.matmul(
                    y_ps[:],
                    lhsT=ctxT_sb[:, qb * 128:(qb + 1) * 128],
                    rhs=wo_sb[:],
                    start=True,
                    stop=True,
                )
                o_t = op.tile([128, D], F32, tag="o", name=f"o_{qb}")
                nc.vector.scalar_tensor_tensor(
                    o_t[:],
                    in0=y_ps[:],
                    scalar=r_sb[:, qb:qb + 1],
                    in1=xb_sb[:, qb, :],
                    op0=mybir.AluOpType.mult,
                    op1=mybir.AluOpType.add,
                )
                store_eng.dma_start(out_ext.ap()[qb * 128:(qb + 1) * 128, :], o_t[:])

            # ctx for half 0, straight through (exp long done)
            ctx0 = ctx_half(0)
            for pair in range(NB // 2):
                ctx_mms(0, pair, ctx0)
            rowsum(0)
            ctx_copy(0, 0, ctx0)
            ctx_copy(0, 1, ctx0)

            # half-0 epilogue interleaved with half-1 ctx accumulation
            ctx1 = ctx_half(1)
            for qb_local in range(8):
                epilogue_qb(0, qb_local, nc.sync if qb_local % 2 == 0 else nc.scalar)
                ctx_mms(1, qb_local, ctx1)
                if qb_local == 0:
                    rowsum(1)

            ctx_copy(1, 0, ctx1)
            ctx_copy(1, 1, ctx1)
            for qb_local in range(8):
                epilogue_qb(1, qb_local, nc.sync if qb_local % 2 == 0 else nc.scalar)


_NC_CACHE = {}


def _get_nc():
    if "nc" not in _NC_CACHE:
        _NC_CACHE["nc"] = build_attention_nc()
    return _NC_CACHE["nc"]


def prep_in_maps(inputs, W_q, W_k, W_v, W_o, b_o):
    """Host-side sharding + layout prep. One batch element per core."""
    B = inputs.shape[0]
    bf = ml_dtypes.bfloat16

    def rearr_w(w):  # [D, U] -> [128, (c u)] with d = c*128 + p
        return np.ascontiguousarray(
            w.reshape(DC, 128, U).transpose(1, 0, 2).reshape(128, DC * U)
        ).astype(bf)

    wq_r = rearr_w(np.asarray(W_q))
    wk_r = rearr_w(np.asarray(W_k))
    wv_r = rearr_w(np.asarray(W_v))
    wo_r = np.ascontiguousarray(np.asarray(W_o)).astype(bf)
    bo = np.asarray(b_o, dtype=np.float32)

    in_maps = []
    for b in range(B):
        xf = np.asarray(inputs[b], dtype=np.float32)
        # xT host layout [p, ns, c, n5]
        xT4 = (
            xf.T.astype(bf)
            .reshape(DC, 128, NS, 512)
            .transpose(1, 2, 0, 3)
            .reshape(128, NS * DC * 512)
        )
        in_maps.append({
            "xb": np.ascontiguousarray(xf + bo),
            "xT": np.ascontiguousarray(xT4),
            "wq": wq_r,
            "wk": wk_r,
            "wv": wv_r,
            "wo": wo_r,
        })
    return in_maps


def kernel(inputs, W_q, W_k, W_v, W_o, b_o):
    in_maps = prep_in_maps(inputs, W_q, W_k, W_v, W_o, b_o)
    nc = _get_nc()
    res = run_bass_kernel_spmd(nc, in_maps, core_ids=list(range(len(in_maps))))
    return np.stack([res.results[i]["out"] for i in range(len(in_maps))], axis=0)


# revision 26
# speedup vs baseline: 1.1629x; 1.1629x over previous
"""Trainium2 Bass kernel for single-head attention + output projection + residual.

Math per batch element b (N=2048, D=512, U=128):
    Q = x @ W_q; K = x @ W_k; V = x @ W_v
    S = Q @ K.T / sqrt(U); A = softmax(S, axis=-1)
    out = (A @ V) @ W_o + b_o + x

Distribution: data-parallel over batch — 8 batch elements, one per NeuronCore.

v3 design:
- Deep software pipeline: all scores of a query-half issued back-to-back,
  ScalarE exp chases through a PSUM pool into fp8 e-tiles (paired [128,2,HQ]).
- ctx accumulation in fp8 DoubleRow: two key-blocks per instruction
  (V pairs [128,2,U] x e pairs), half the bf16 tensor cycles.
- Softmax denominator on the tensor engine: DoubleRow ones-matmuls against
  the same fp8 e-pairs accumulate den[1,q] in PSUM (no DVE Esum chain at
  all); r = 1/den recovered via a tiny PSUM->SBUF DMA + PE transposes.
- exp has a folded -1 bias shift (cancels in normalization) keeping fp8e4
  in range; bias+residual folded host-side (xb = x + b_o).
- den-h0 matmuls hide in the sH1 score phase's tensor slack; den-h1 pairs
  ride along with ctx-h1 pairs (identical exp dependencies).

Softmax max-subtraction is skipped: scores/sqrt(U) are bounded (~±6) for any
well-scaled input; exp output with -1 shift stays under fp8e4's ±240 range.
"""

import numpy as np
import ml_dtypes

import concourse.bass as bass
import concourse.tile as tile
from concourse import bacc, mybir
from concourse.bass_utils import run_bass_kernel_spmd

N = 2048
D = 512
U = 128
NB = N // 128  # 16 query/key blocks
DC = D // 128  # 4 d-chunks
NS = N // 512  # 4 free-dim slices of 512
HQ = N // 2  # queries per half
NP = NB // 2  # 8 key-block pairs
INV_SQRT_U = 1.0 / np.sqrt(U)
EXP_SHIFT = -1.0  # exp(s - 1): cancels in normalization, keeps fp8e4 in range

F32 = mybir.dt.float32
BF16 = mybir.dt.bfloat16
FP8 = mybir.dt.float8e4

DR = mybir.MatmulPerfMode.DoubleRow


def build_attention_nc():
    nc = bacc.Bacc("TRN2", target_bir_lowering=False, debug=False)

    xb_ext = nc.declare_dram_parameter("xb", [N, D], F32, isOutput=False)
    # host layout [p, ns, c, n5]: per-partition 4KB contiguous runs per piece
    xT_ext = nc.declare_dram_parameter("xT", [128, NS * DC * 512], BF16, isOutput=False)
    wq_ext = nc.declare_dram_parameter("wq", [128, D], BF16, isOutput=False)
    wk_ext = nc.declare_dram_parameter("wk", [128, D], BF16, isOutput=False)
    wv_ext = nc.declare_dram_parameter("wv", [128, D], BF16, isOutput=False)
    wo_ext = nc.declare_dram_parameter("wo", [U, D], BF16, isOutput=False)
    out_ext = nc.declare_dram_parameter("out", [N, D], F32, isOutput=True)

    with tile.TileContext(nc) as tc:
        _build_body(nc, tc, xb_ext, xT_ext, wq_ext, wk_ext, wv_ext, wo_ext, out_ext)
    nc.compile()
    return nc


def _build_body(nc, tc, xb_ext, xT_ext, wq_ext, wk_ext, wv_ext, wo_ext, out_ext):
    from contextlib import ExitStack

    with ExitStack() as ctx:
        const = ctx.enter_context(tc.tile_pool(name="const", bufs=1))

        # ---- loads: weights first (they gate the first matmuls) ----
        wq_sb = const.tile([128, D], BF16)  # [d-within-chunk, (c u)]
        wk_sb = const.tile([128, D], BF16)
        wv_sb = const.tile([128, D], BF16)
        wo_sb = const.tile([U, D], BF16)
        nc.scalar.dma_start(wk_sb[:], wk_ext.ap())
        nc.scalar.dma_start(wq_sb[:], wq_ext.ap())
        nc.gpsimd.dma_start(wv_sb[:], wv_ext.ap())
        nc.gpsimd.dma_start(wo_sb[:], wo_ext.ap())

        ones8_sb = const.tile([128, 2, 32], FP8)
        nc.vector.memset(ones8_sb[:], 1.0)
        ident_sb = const.tile([1, 1], F32)
        nc.vector.memset(ident_sb[:], 1.0)
        eshift_sb = const.tile([128, 1], F32)
        nc.vector.memset(eshift_sb[:], EXP_SHIFT)
        # force the exp activation table load while DMAs are in flight
        scratch = const.tile([128, 1], F32)
        nc.scalar.activation(
            scratch[:], eshift_sb[:], mybir.ActivationFunctionType.Exp, scale=1.0
        )

        # xT in SBUF as [p, ns, c, n5]; 4 ns-pieces with 4KB/partition runs
        xT_sb = const.tile([128, NS, DC, 512], BF16)
        xT_r = xT_ext.ap().rearrange("p (ns c n) -> p ns c n", ns=NS, c=DC)
        for ns in range(NS):
            nc.sync.dma_start(xT_sb[:, ns], xT_r[:, ns])

        # xb = x + b_o precomputed on host; needed only for the epilogue.
        xb_sb = const.tile([128, NB, D], F32)
        xb_r = xb_ext.ap().rearrange("(nb p) d -> p nb d", p=128)
        nc.sync.dma_start(xb_sb[0:64], xb_r[0:64])
        nc.gpsimd.dma_start(xb_sb[64:128], xb_r[64:128])

        QT_sb = const.tile([U, N], BF16)
        KT_sb = const.tile([U, N], BF16)
        V_sb = const.tile([128, NB, U], FP8)  # kb-pair p at [:, 2p:2p+2, :]
        ctxT_sb = const.tile([U, N], BF16)
        den_sb = [const.tile([1, HQ], F32, name=f"den_sb_{h}") for h in range(2)]
        r_sb = const.tile([128, NB], F32)

        # den accumulators: DR matmul dst must sit at partition base 0, so
        # each (h, j) accumulator gets its own bank ([32, 512] tile, base 0).
        # bufs=2 rotates: h1's tiles reuse h0's banks after den_finalize(0).
        # DR ldweights needs weight-tile stride %16==0 and a 32-partition dst:
        # 32 ones columns -> 32 identical den rows; row 0 is the one read.
        den_pool = ctx.enter_context(
            tc.tile_pool(name="den_ps", bufs=2, space="PSUM")
        )
        den_ps = {}

        def proj_slice(pool, w_sb, oT, ns):
            ps = pool.tile([128, 512], F32, tag="ps", name=f"pp_{oT.tensor.name}_{ns}")
            for c in range(DC):
                nc.tensor.matmul(
                    ps[:],
                    lhsT=w_sb[:, c * 128:(c + 1) * 128],
                    rhs=xT_sb[:, ns, c, :],
                    start=(c == 0),
                    stop=(c == DC - 1),
                )
            nc.vector.tensor_copy(oT[:, ns * 512:(ns + 1) * 512], ps[:])

        def make_v(pool, g):
            # 4 key-blocks' V projections into one PSUM tile, one fp8 copy out
            ps = pool.tile([128, 512], F32, tag="ps", name=f"v_{g}")
            for q in range(4):
                kb = g * 4 + q
                ns, n0 = divmod(kb * 128, 512)
                for c in range(DC):
                    nc.tensor.matmul(
                        ps[:, q * 128:(q + 1) * 128],
                        lhsT=xT_sb[:, ns, c, n0:n0 + 128],
                        rhs=wv_sb[:, c * 128:(c + 1) * 128],
                        start=(c == 0),
                        stop=(c == DC - 1),
                        skip_group_check=True,
                    )
            nc.vector.tensor_copy(V_sb[:, g * 4:(g + 1) * 4, :], ps[:])

        e_t = [None] * NB  # paired e-tiles [128, 2, HQ], 8 per half

        def den_mms(h, pr):
            for j in range(2):
                if pr == 0:
                    den_ps[(h, j)] = den_pool.tile(
                        [32, 512], F32, tag="den", name=f"den_{h}_{j}"
                    )
                nc.tensor.matmul(
                    den_ps[(h, j)][:],
                    lhsT=ones8_sb[:],
                    rhs=e_t[h * NP + pr][:, :, j * 512:(j + 1) * 512],
                    start=(pr == 0),
                    stop=(pr == NP - 1),
                    perf_mode=DR,
                    skip_group_check=True,
                )

        # ---- phase 1: projections + all scores/exp + den-h0 ----
        with (
            tc.tile_pool(name="proj_ps", bufs=2, space="PSUM") as pp,
            tc.tile_pool(name="s_ps", bufs=2, space="PSUM") as sp,
            tc.tile_pool(name="e_sb", bufs=16) as ep,
        ):
            proj_slice(pp, wk_sb, KT_sb, 0)
            proj_slice(pp, wq_sb, QT_sb, 0)
            proj_slice(pp, wq_sb, QT_sb, 1)

            def scores_block(h, kb):
                pr = (h * NB + kb) // 2
                t = kb % 2
                q0 = h * HQ
                if t == 0:
                    e_t[pr] = ep.tile([128, 2, HQ], FP8, tag="e", name=f"e_{h}_{kb}")
                s_ps = sp.tile([128, HQ], F32, tag="s", name=f"s_{h}_{kb}")
                for j in range(2):
                    nc.tensor.matmul(
                        s_ps[:, j * 512:(j + 1) * 512],
                        lhsT=KT_sb[:, kb * 128:(kb + 1) * 128],
                        rhs=QT_sb[:, q0 + j * 512:q0 + (j + 1) * 512],
                        start=True,
                        stop=True,
                    )
                nc.scalar.activation(
                    e_t[pr][:, t, :],
                    s_ps[:],
                    mybir.ActivationFunctionType.Exp,
                    bias=eshift_sb[:],
                    scale=INV_SQRT_U,
                )

            for kb in range(4):
                scores_block(0, kb)
            proj_slice(pp, wk_sb, KT_sb, 1)
            for kb in range(4, 8):
                scores_block(0, kb)
            proj_slice(pp, wk_sb, KT_sb, 2)
            for kb in range(8, 12):
                scores_block(0, kb)
            proj_slice(pp, wk_sb, KT_sb, 3)
            for kb in range(12, 16):
                scores_block(0, kb)

            proj_slice(pp, wq_sb, QT_sb, 2)
            proj_slice(pp, wq_sb, QT_sb, 3)
            for g in range(4):
                make_v(pp, g)

            # half-1 scores with den-h0 matmuls hidden in the tensor slack
            for kb in range(NB):
                scores_block(1, kb)
                if kb % 2 == 1:
                    den_mms(0, kb // 2)

        # ---- phase 2: fp8 DoubleRow ctx + den-h1 + epilogue ----
        with (
            tc.tile_pool(name="ctx_ps", bufs=2, space="PSUM") as cp,
            tc.tile_pool(name="d_ps", bufs=2, space="PSUM") as dp,
            tc.tile_pool(name="o_sb", bufs=4) as op,
        ):
            def ctx_half(h):
                return [
                    cp.tile([U, 512], F32, tag="ctx", name=f"ctx_ps_{h}_{j}")
                    for j in range(2)
                ]

            def ctx_mms(h, pair, ctx_ps):
                pr = h * NP + pair
                v2 = V_sb[:, 2 * pair:2 * pair + 2, :]
                for j in range(2):
                    nc.tensor.matmul(
                        ctx_ps[j][:],
                        lhsT=v2,
                        rhs=e_t[pr][:, :, j * 512:(j + 1) * 512],
                        start=(pair == 0),
                        stop=(pair == NP - 1),
                        perf_mode=DR,
                    )

            def den_finalize(h):
                # PSUM [1,512]x2 -> SBUF, then 8 PE transposes -> [128,8] -> 1/x
                for j in range(2):
                    nc.vector.tensor_copy(
                        den_sb[h][:, j * 512:(j + 1) * 512], den_ps[(h, j)][0:1, :]
                    )
                rT = dp.tile([128, 512], F32, tag="d", name=f"rT_{h}")
                for i in range(8):
                    nc.tensor.matmul(
                        rT[:, i:i + 1],
                        lhsT=den_sb[h][:, i * 128:(i + 1) * 128],
                        rhs=ident_sb[:],
                        is_transpose=True,
                        skip_group_check=True,
                    )
                nc.vector.reciprocal(r_sb[:, h * 8:h * 8 + 8], rT[:, 0:8])

            def ctx_copy(h, j, ctx_ps):
                nc.vector.tensor_copy(
                    ctxT_sb[:, h * HQ + j * 512:h * HQ + (j + 1) * 512], ctx_ps[j][:]
                )

            def epilogue_qb(h, qb_local, store_eng):
                qb = h * 8 + qb_local
                y_ps = dp.tile([128, D], F32, tag="d", name=f"y_{qb}")
                nc.tensor.matmul(
                    y_ps[:],
                    lhsT=ctxT_sb[:, qb * 128:(qb + 1) * 128],
                    rhs=wo_sb[:],
                    start=True,
                    stop=True,
                )
                o_t = op.tile([128, D], F32, tag="o", name=f"o_{qb}")
                nc.vector.scalar_tensor_tensor(
                    o_t[:],
                    in0=y_ps[:],
                    scalar=r_sb[:, qb:qb + 1],
                    in1=xb_sb[:, qb, :],
                    op0=mybir.AluOpType.mult,
                    op1=mybir.AluOpType.add,
                )
                store_eng.dma_start(out_ext.ap()[qb * 128:(qb + 1) * 128, :], o_t[:])

            # ctx for half 0, straight through (exp long done)
            ctx0 = ctx_half(0)
            for pair in range(NP):
                ctx_mms(0, pair, ctx0)
            den_finalize(0)
            ctx_copy(0, 0, ctx0)
            ctx_copy(0, 1, ctx0)

            # half-0 epilogue interleaved with half-1 ctx+den accumulation
            ctx1 = ctx_half(1)
            for qb_local in range(8):
                epilogue_qb(0, qb_local, nc.sync if qb_local % 2 == 0 else nc.scalar)
                den_mms(1, qb_local)
                ctx_mms(1, qb_local, ctx1)

            den_finalize(1)
            ctx_copy(1, 0, ctx1)
            ctx_copy(1, 1, ctx1)
            for qb_local in range(8):
                epilogue_qb(1, qb_local, nc.sync if qb_local % 2 == 0 else nc.scalar)


_NC_CACHE = {}


def _get_nc():
    if "nc" not in _NC_CACHE:
        _NC_CACHE["nc"] = build_attention_nc()
    return _NC_CACHE["nc"]


def prep_in_maps(inputs, W_q, W_k, W_v, W_o, b_o):
    """Host-side sharding + layout prep. One batch element per core."""
    B = inputs.shape[0]
    bf = ml_dtypes.bfloat16

    def rearr_w(w):  # [D, U] -> [128, (c u)] with d = c*128 + p
        return np.ascontiguousarray(
            w.reshape(DC, 128, U).transpose(1, 0, 2).reshape(128, DC * U)
        ).astype(bf)

    wq_r = rearr_w(np.asarray(W_q))
    wk_r = rearr_w(np.asarray(W_k))
    wv_r = rearr_w(np.asarray(W_v))
    wo_r = np.ascontiguousarray(np.asarray(W_o)).astype(bf)
    bo = np.asarray(b_o, dtype=np.float32)

    in_maps = []
    for b in range(B):
        xf = np.asarray(inputs[b], dtype=np.float32)
        # xT host layout [p, ns, c, n5]
        xT4 = (
            xf.T.astype(bf)
            .reshape(DC, 128, NS, 512)
            .transpose(1, 2, 0, 3)
            .reshape(128, NS * DC * 512)
        )
        in_maps.append({
            "xb": np.ascontiguousarray(xf + bo),
            "xT": np.ascontiguousarray(xT4),
            "wq": wq_r,
            "wk": wk_r,
            "wv": wv_r,
            "wo": wo_r,
        })
    return in_maps


def kernel(inputs, W_q, W_k, W_v, W_o, b_o):
    in_maps = prep_in_maps(inputs, W_q, W_k, W_v, W_o, b_o)
    nc = _get_nc()
    res = run_bass_kernel_spmd(nc, in_maps, core_ids=list(range(len(in_maps))))
    return np.stack([res.results[i]["out"] for i in range(len(in_maps))], axis=0)


# revision 27
# speedup vs baseline: 1.1680x; 1.0044x over previous
"""Trainium2 Bass kernel for single-head attention + output projection + residual.

Math per batch element b (N=2048, D=512, U=128):
    Q = x @ W_q; K = x @ W_k; V = x @ W_v
    S = Q @ K.T / sqrt(U); A = softmax(S, axis=-1)
    out = (A @ V) @ W_o + b_o + x

Distribution: data-parallel over batch — 8 batch elements, one per NeuronCore.

v3 design:
- Deep software pipeline: all scores of a query-half issued back-to-back,
  ScalarE exp chases through a PSUM pool into fp8 e-tiles (paired [128,2,HQ]).
- ctx accumulation in fp8 DoubleRow: two key-blocks per instruction
  (V pairs [128,2,U] x e pairs), half the bf16 tensor cycles.
- Softmax denominator on the tensor engine: DoubleRow ones-matmuls against
  the same fp8 e-pairs accumulate den[1,q] in PSUM (no DVE Esum chain at
  all); r = 1/den recovered via a tiny PSUM->SBUF DMA + PE transposes.
- exp has a folded -1 bias shift (cancels in normalization) keeping fp8e4
  in range; bias+residual folded host-side (xb = x + b_o).
- den-h0 matmuls hide in the sH1 score phase's tensor slack; den-h1 pairs
  ride along with ctx-h1 pairs (identical exp dependencies).

Softmax max-subtraction is skipped: scores/sqrt(U) are bounded (~±6) for any
well-scaled input; exp output with -1 shift stays under fp8e4's ±240 range.
"""

import numpy as np
import ml_dtypes

import concourse.bass as bass
import concourse.tile as tile
from concourse import bacc, mybir
from concourse.bass_utils import run_bass_kernel_spmd

N = 2048
D = 512
U = 128
NB = N // 128  # 16 query/key blocks
DC = D // 128  # 4 d-chunks
NS = N // 512  # 4 free-dim slices of 512
HQ = N // 2  # queries per half
NP = NB // 2  # 8 key-block pairs
INV_SQRT_U = 1.0 / np.sqrt(U)
EXP_SHIFT = -1.0  # exp(s - 1): cancels in normalization, keeps fp8e4 in range

F32 = mybir.dt.float32
BF16 = mybir.dt.bfloat16
FP8 = mybir.dt.float8e4

DR = mybir.MatmulPerfMode.DoubleRow


def build_attention_nc():
    nc = bacc.Bacc("TRN2", target_bir_lowering=False, debug=False)

    xb_ext = nc.declare_dram_parameter("xb", [N, D], F32, isOutput=False)
    # host layout [p, ns, c, n5]: per-partition 4KB contiguous runs per piece
    xT_ext = nc.declare_dram_parameter("xT", [128, NS * DC * 512], BF16, isOutput=False)
    wq_ext = nc.declare_dram_parameter("wq", [128, D], BF16, isOutput=False)
    wk_ext = nc.declare_dram_parameter("wk", [128, D], BF16, isOutput=False)
    wv_ext = nc.declare_dram_parameter("wv", [128, D], BF16, isOutput=False)
    wo_ext = nc.declare_dram_parameter("wo", [U, D], BF16, isOutput=False)
    out_ext = nc.declare_dram_parameter("out", [N, D], F32, isOutput=True)

    with tile.TileContext(nc) as tc:
        _build_body(nc, tc, xb_ext, xT_ext, wq_ext, wk_ext, wv_ext, wo_ext, out_ext)
    nc.compile()
    return nc


def _build_body(nc, tc, xb_ext, xT_ext, wq_ext, wk_ext, wv_ext, wo_ext, out_ext):
    from contextlib import ExitStack

    with ExitStack() as ctx:
        const = ctx.enter_context(tc.tile_pool(name="const", bufs=1))

        # ---- loads: weights first (they gate the first matmuls) ----
        wq_sb = const.tile([128, D], BF16)  # [d-within-chunk, (c u)]
        wk_sb = const.tile([128, D], BF16)
        wv_sb = const.tile([128, D], BF16)
        wo_sb = const.tile([U, D], BF16)
        nc.scalar.dma_start(wk_sb[:], wk_ext.ap())
        nc.scalar.dma_start(wq_sb[:], wq_ext.ap())
        nc.gpsimd.dma_start(wv_sb[:], wv_ext.ap())
        nc.gpsimd.dma_start(wo_sb[:], wo_ext.ap())

        ones8_sb = const.tile([128, 2, 32], FP8)
        nc.vector.memset(ones8_sb[:], 1.0)
        ident_sb = const.tile([1, 1], F32)
        nc.vector.memset(ident_sb[:], 1.0)
        eshift_sb = const.tile([128, 1], F32)
        nc.vector.memset(eshift_sb[:], EXP_SHIFT)
        # force the exp activation table load while DMAs are in flight
        scratch = const.tile([128, 1], F32)
        nc.scalar.activation(
            scratch[:], eshift_sb[:], mybir.ActivationFunctionType.Exp, scale=1.0
        )

        # xT in SBUF as [p, ns, c, n5]; 4 ns-pieces with 4KB/partition runs,
        # each split across two queues by partition half for 2x arrival rate
        xT_sb = const.tile([128, NS, DC, 512], BF16)
        xT_r = xT_ext.ap().rearrange("p (ns c n) -> p ns c n", ns=NS, c=DC)
        for ns in range(NS):
            nc.sync.dma_start(xT_sb[0:64, ns], xT_r[0:64, ns])
            nc.scalar.dma_start(xT_sb[64:128, ns], xT_r[64:128, ns])

        # xb = x + b_o precomputed on host; needed only for the epilogue.
        xb_sb = const.tile([128, NB, D], F32)
        xb_r = xb_ext.ap().rearrange("(nb p) d -> p nb d", p=128)
        nc.sync.dma_start(xb_sb[0:64], xb_r[0:64])
        nc.gpsimd.dma_start(xb_sb[64:128], xb_r[64:128])

        QT_sb = const.tile([U, N], BF16)
        KT_sb = const.tile([U, N], BF16)
        V_sb = const.tile([128, NB, U], FP8)  # kb-pair p at [:, 2p:2p+2, :]
        ctxT_sb = const.tile([U, N], BF16)
        den_sb = [const.tile([1, HQ], F32, name=f"den_sb_{h}") for h in range(2)]
        r_sb = const.tile([128, NB], F32)

        # den accumulators: DR matmul dst must sit at partition base 0, so
        # each (h, j) accumulator gets its own bank ([32, 512] tile, base 0).
        # bufs=2 rotates: h1's tiles reuse h0's banks after den_finalize(0).
        # DR ldweights needs weight-tile stride %16==0 and a 32-partition dst:
        # 32 ones columns -> 32 identical den rows; row 0 is the one read.
        den_pool = ctx.enter_context(
            tc.tile_pool(name="den_ps", bufs=2, space="PSUM")
        )
        den_ps = {}

        def proj_slice(pool, w_sb, oT, ns):
            ps = pool.tile([128, 512], F32, tag="ps", name=f"pp_{oT.tensor.name}_{ns}")
            for c in range(DC):
                nc.tensor.matmul(
                    ps[:],
                    lhsT=w_sb[:, c * 128:(c + 1) * 128],
                    rhs=xT_sb[:, ns, c, :],
                    start=(c == 0),
                    stop=(c == DC - 1),
                )
            nc.vector.tensor_copy(oT[:, ns * 512:(ns + 1) * 512], ps[:])

        def make_v(pool, g):
            # 4 key-blocks' V projections into one PSUM tile, one fp8 copy out
            ps = pool.tile([128, 512], F32, tag="ps", name=f"v_{g}")
            for q in range(4):
                kb = g * 4 + q
                ns, n0 = divmod(kb * 128, 512)
                for c in range(DC):
                    nc.tensor.matmul(
                        ps[:, q * 128:(q + 1) * 128],
                        lhsT=xT_sb[:, ns, c, n0:n0 + 128],
                        rhs=wv_sb[:, c * 128:(c + 1) * 128],
                        start=(c == 0),
                        stop=(c == DC - 1),
                        skip_group_check=True,
                    )
            nc.vector.tensor_copy(V_sb[:, g * 4:(g + 1) * 4, :], ps[:])

        e_t = [None] * NB  # paired e-tiles [128, 2, HQ], 8 per half

        def den_mms(h, pr):
            for j in range(2):
                if pr == 0:
                    den_ps[(h, j)] = den_pool.tile(
                        [32, 512], F32, tag="den", name=f"den_{h}_{j}"
                    )
                nc.tensor.matmul(
                    den_ps[(h, j)][:],
                    lhsT=ones8_sb[:],
                    rhs=e_t[h * NP + pr][:, :, j * 512:(j + 1) * 512],
                    start=(pr == 0),
                    stop=(pr == NP - 1),
                    perf_mode=DR,
                    skip_group_check=True,
                )

        # ---- phase 1: projections + all scores/exp + den-h0 ----
        with (
            tc.tile_pool(name="proj_ps", bufs=2, space="PSUM") as pp,
            tc.tile_pool(name="s_ps", bufs=2, space="PSUM") as sp,
            tc.tile_pool(name="e_sb", bufs=16) as ep,
        ):
            proj_slice(pp, wk_sb, KT_sb, 0)
            proj_slice(pp, wq_sb, QT_sb, 0)
            proj_slice(pp, wq_sb, QT_sb, 1)

            def scores_block(h, kb):
                pr = (h * NB + kb) // 2
                t = kb % 2
                q0 = h * HQ
                if t == 0:
                    e_t[pr] = ep.tile([128, 2, HQ], FP8, tag="e", name=f"e_{h}_{kb}")
                s_ps = sp.tile([128, HQ], F32, tag="s", name=f"s_{h}_{kb}")
                for j in range(2):
                    nc.tensor.matmul(
                        s_ps[:, j * 512:(j + 1) * 512],
                        lhsT=KT_sb[:, kb * 128:(kb + 1) * 128],
                        rhs=QT_sb[:, q0 + j * 512:q0 + (j + 1) * 512],
                        start=True,
                        stop=True,
                    )
                nc.scalar.activation(
                    e_t[pr][:, t, :],
                    s_ps[:],
                    mybir.ActivationFunctionType.Exp,
                    bias=eshift_sb[:],
                    scale=INV_SQRT_U,
                )

            for kb in range(4):
                scores_block(0, kb)
            proj_slice(pp, wk_sb, KT_sb, 1)
            for kb in range(4, 8):
                scores_block(0, kb)
            proj_slice(pp, wk_sb, KT_sb, 2)
            for kb in range(8, 12):
                scores_block(0, kb)
            proj_slice(pp, wk_sb, KT_sb, 3)
            for kb in range(12, 16):
                scores_block(0, kb)

            proj_slice(pp, wq_sb, QT_sb, 2)
            proj_slice(pp, wq_sb, QT_sb, 3)
            for g in range(4):
                make_v(pp, g)

            # half-1 scores with den-h0 matmuls hidden in the tensor slack
            for kb in range(NB):
                scores_block(1, kb)
                if kb % 2 == 1:
                    den_mms(0, kb // 2)

        # ---- phase 2: fp8 DoubleRow ctx + den-h1 + epilogue ----
        with (
            tc.tile_pool(name="ctx_ps", bufs=2, space="PSUM") as cp,
            tc.tile_pool(name="d_ps", bufs=2, space="PSUM") as dp,
            tc.tile_pool(name="o_sb", bufs=4) as op,
        ):
            def ctx_half(h):
                return [
                    cp.tile([U, 512], F32, tag="ctx", name=f"ctx_ps_{h}_{j}")
                    for j in range(2)
                ]

            def ctx_mms(h, pair, ctx_ps):
                pr = h * NP + pair
                v2 = V_sb[:, 2 * pair:2 * pair + 2, :]
                for j in range(2):
                    nc.tensor.matmul(
                        ctx_ps[j][:],
                        lhsT=v2,
                        rhs=e_t[pr][:, :, j * 512:(j + 1) * 512],
                        start=(pair == 0),
                        stop=(pair == NP - 1),
                        perf_mode=DR,
                    )

            def den_finalize(h):
                # PSUM [1,512]x2 -> SBUF, then 8 PE transposes -> [128,8] -> 1/x
                for j in range(2):
                    nc.vector.tensor_copy(
                        den_sb[h][:, j * 512:(j + 1) * 512], den_ps[(h, j)][0:1, :]
                    )
                rT = dp.tile([128, 512], F32, tag="d", name=f"rT_{h}")
                for i in range(8):
                    nc.tensor.matmul(
                        rT[:, i:i + 1],
                        lhsT=den_sb[h][:, i * 128:(i + 1) * 128],
                        rhs=ident_sb[:],
                        is_transpose=True,
                        skip_group_check=True,
                    )
                nc.vector.reciprocal(r_sb[:, h * 8:h * 8 + 8], rT[:, 0:8])

            def ctx_copy(h, j, ctx_ps):
                nc.vector.tensor_copy(
                    ctxT_sb[:, h * HQ + j * 512:h * HQ + (j + 1) * 512], ctx_ps[j][:]
                )

            def epilogue_qb(h, qb_local, store_eng):
                qb = h * 8 + qb_local
                y_ps = dp.tile([128, D], F32, tag="d", name=f"y_{qb}")
                nc.tensor.matmul(
                    y_ps[:],
                    lhsT=ctxT_sb[:, qb * 128:(qb + 1) * 128],
                    rhs=wo_sb[:],
                    start=True,
                    stop=True,
                )
                o_t = op.tile([128, D], F32, tag="o", name=f"o_{qb}")
                nc.vector.scalar_tensor_tensor(
                    o_t[:],
                    in0=y_ps[:],
                    scalar=r_sb[:, qb:qb + 1],
                    in1=xb_sb[:, qb, :],
                    op0=mybir.AluOpType.mult,
                    op1=mybir.AluOpType.add,
                )
                store_eng.dma_start(out_ext.ap()[qb * 128:(qb + 1) * 128, :], o_t[:])

            # ctx for half 0, straight through (exp long done)
            ctx0 = ctx_half(0)
            for pair in range(NP):
                ctx_mms(0, pair, ctx0)
            den_finalize(0)
            ctx_copy(0, 0, ctx0)
            ctx_copy(0, 1, ctx0)

            # half-0 epilogue interleaved with half-1 ctx+den accumulation
            ctx1 = ctx_half(1)
            for qb_local in range(8):
                epilogue_qb(0, qb_local, nc.sync if qb_local % 2 == 0 else nc.scalar)
                den_mms(1, qb_local)
                ctx_mms(1, qb_local, ctx1)

            den_finalize(1)
            ctx_copy(1, 0, ctx1)
            ctx_copy(1, 1, ctx1)
            for qb_local in range(8):
                epilogue_qb(1, qb_local, nc.sync if qb_local % 2 == 0 else nc.scalar)


_NC_CACHE = {}


def _get_nc():
    if "nc" not in _NC_CACHE:
        _NC_CACHE["nc"] = build_attention_nc()
    return _NC_CACHE["nc"]


def prep_in_maps(inputs, W_q, W_k, W_v, W_o, b_o):
    """Host-side sharding + layout prep. One batch element per core."""
    B = inputs.shape[0]
    bf = ml_dtypes.bfloat16

    def rearr_w(w):  # [D, U] -> [128, (c u)] with d = c*128 + p
        return np.ascontiguousarray(
            w.reshape(DC, 128, U).transpose(1, 0, 2).reshape(128, DC * U)
        ).astype(bf)

    wq_r = rearr_w(np.asarray(W_q))
    wk_r = rearr_w(np.asarray(W_k))
    wv_r = rearr_w(np.asarray(W_v))
    wo_r = np.ascontiguousarray(np.asarray(W_o)).astype(bf)
    bo = np.asarray(b_o, dtype=np.float32)

    in_maps = []
    for b in range(B):
        xf = np.asarray(inputs[b], dtype=np.float32)
        # xT host layout [p, ns, c, n5]
        xT4 = (
            xf.T.astype(bf)
            .reshape(DC, 128, NS, 512)
            .transpose(1, 2, 0, 3)
            .reshape(128, NS * DC * 512)
        )
        in_maps.append({
            "xb": np.ascontiguousarray(xf + bo),
            "xT": np.ascontiguousarray(xT4),
            "wq": wq_r,
            "wk": wk_r,
            "wv": wv_r,
            "wo": wo_r,
        })
    return in_maps


def kernel(inputs, W_q, W_k, W_v, W_o, b_o):
    in_maps = prep_in_maps(inputs, W_q, W_k, W_v, W_o, b_o)
    nc = _get_nc()
    res = run_bass_kernel_spmd(nc, in_maps, core_ids=list(range(len(in_maps))))
    return np.stack([res.results[i]["out"] for i in range(len(in_maps))], axis=0)
